# revision 32
# baseline (speedup 1.0000x reference)
"""Trainium2 Bass kernel for nn_MatrixLSTMCell (mLSTM, parallel stabilized form).

Sharding: 48 (b, head) pairs across 8 cores -> each core handles one batch b
and a group of 6 heads (2 cores per batch).

Algorithm (equivalent to the reference up to fp tolerance):
  lp[t]  = cumsum(softplus(-fg))[t]      (= -L[t])
  m[t]   = ig[t] + lp[t];  cH = max_t m  (per head)
  em[j]  = exp(m[j] - cH)
  g[i]   = exp(lp[i] - cH + ln 8)        (ln 8 = ln sqrt(dh))
  U[i]   = sum_{j<=i} (q_i.k_j) em[j] [v|1|vsum][j]   (chunked linear attn)
  R[i]   = U's ones-column (rowsum of weights), sumU = U's vsum-column
  h[i]   = U / max(|R|, g)               (exactly the reference normalizer:
           the global 1/sqrt(dh) and the per-row exp(cH - M[i]) factors
           cancel inside max(), and the +eps term is provably negligible)
  out    = (h - mean) / sqrt(var + 1e-5) per (i, head) over dh=64
The ones/vsum columns make rowsum and mean free (matmul byproducts).
The causal sum splits into 128-row chunks: tril-masked qk tiles (staged in
SBUF during the input-DMA window) times (va*em), plus a running state
W = sum_j k[j] em[j] va[j]^T applied as q @ W.
"""

import math
import os

import numpy as np
import ml_dtypes

import concourse.bass as bass
import concourse.bacc as bacc
import concourse.mybir as mybir
import concourse.tile as tile
from concourse.bass_utils import run_bass_kernel_spmd

F32 = mybir.dt.float32
BF16 = mybir.dt.bfloat16

B, S, DIM = 4, 1024, 768
NH, DH = 12, 64
HPC = 6            # heads per core
GD = HPC * DH      # 384 output dims per core
DA = DH + 2        # v augmented with ones and vsum columns
NCH = S // 128     # 8 chunks
EPS_NORM = 1e-5
LN8 = math.log(8.0)
AF = mybir.ActivationFunctionType
OP = mybir.AluOpType


def build_nc():
    # Bacc (not raw Bass): its compile() pass splits multi-sem waits into
    # standalone event-semaphore instructions (TRN2 allows 1 wait/instr).
    nc = bacc.Bacc(None, target_bir_lowering=False)

    xt = nc.dram_tensor("xt", [3 * DIM, S], BF16, kind="ExternalInput")[:]
    kn = nc.dram_tensor("kn", [S, GD], BF16, kind="ExternalInput")[:]
    vn = nc.dram_tensor("vn", [S, HPC * DA], BF16, kind="ExternalInput")[:]
    wt = nc.dram_tensor("wt", [128, 18 * 2 * HPC], BF16, kind="ExternalInput")[:]
    bias = nc.dram_tensor("bias", [2 * HPC, 1], F32, kind="ExternalInput")[:]
    lmask = nc.dram_tensor("lmask", [48, 48], F32, kind="ExternalInput")[:]
    bc6 = nc.dram_tensor("bc6", [HPC, 48], F32, kind="ExternalInput")[:]
    out = nc.dram_tensor("out", [S, GD], BF16, kind="ExternalOutput")[:]

    with tile.TileContext(nc) as tc:
        with tc.tile_pool(name="persist", bufs=1) as persist:
            _body(nc, tc, persist, xt, kn, vn, wt, bias, lmask, bc6, out)
    nc.finalize()
    return nc


def _body(nc, tc, persist, xt, kn, vn, wt, bias, lmask, bc6, out):
    # ---------------- persistent SBUF ----------------
    xt_sb = persist.tile([128, 18, S], BF16)
    kn_sb = persist.tile([128, NCH, GD], BF16)
    vn_sb = persist.tile([128, NCH, HPC * DA], BF16)
    wt_sb = persist.tile([128, 18, 2 * HPC], BF16)
    bias_sb = persist.tile([2 * HPC, 1], F32)
    lmask_sb = persist.tile([48, 48], F32)
    bc6_sb = persist.tile([HPC, 48], F32)
    mask_sb = persist.tile([128, 128], BF16)     # 1.0 where j<=i else 0
    ident48 = persist.tile([48, 48], F32)
    ident1 = persist.tile([1, 1], F32)
    touch_sb = persist.tile([128, 8], F32)
    epsn_sb = persist.tile([128, 1], F32)
    zrow_sb = persist.tile([1, HPC * DA], BF16)
    zcol_sb = persist.tile([1, 128], BF16)

    g12_sb = persist.tile([2 * HPC, S], F32)     # rows 0-5 fg, 6-11 ig
    fg48 = persist.tile([48, 128], F32)          # partition h*8+c
    ig48 = persist.tile([48, 128], F32)
    ef48 = persist.tile([48, 128], F32)
    sp48 = persist.tile([48, 128], F32)
    ic48 = persist.tile([48, 128], F32)          # intra-chunk cumsum
    lp48 = persist.tile([48, 128], F32)
    m48 = persist.tile([48, 128], F32)
    rmax = persist.tile([48, 1], F32)
    mx6 = persist.tile([1, HPC], F32)
    cH6 = persist.tile([HPC, 1], F32)
    negb = persist.tile([48, 1], F32)            # -cH per (h,c) partition
    negb8 = persist.tile([48, 1], F32)           # -cH + ln 8
    em48 = persist.tile([48, 128], F32)
    g48 = persist.tile([48, 128], F32)
    em_col = persist.tile([128, 48], BF16)       # [i, c*6+h]
    g_col = persist.tile([128, 48], F32)

    cp0_sb = persist.tile([128, NCH, HPC, 128], BF16)   # tril(qk) staged

    xt_c = xt.rearrange("(c p) s -> c p s", p=128)
    kn_c = kn.rearrange("(r p) d -> p r d", p=128)
    vn_c = vn.rearrange("(r p) d -> p r d", p=128)

    # ---------------- loads (wt/bias first: gates need them) ----------------
    nc.sync.dma_start(out=wt_sb[:], in_=wt.rearrange("p (c j) -> p c j", c=18))
    nc.sync.dma_start(out=bias_sb[:], in_=bias)
    nc.sync.dma_start(out=lmask_sb[:], in_=lmask)
    nc.sync.dma_start(out=bc6_sb[:], in_=bc6)
    for c in range(18):
        nc.sync.dma_start(out=xt_sb[:, c, :], in_=xt_c[c])
    nc.sync.dma_start(out=kn_sb[:], in_=kn_c)
    nc.sync.dma_start(out=vn_sb[:], in_=vn_c)

    # ---------------- constants ----------------
    nc.gpsimd.memset(mask_sb[:], 0.0)
    nc.gpsimd.affine_select(
        out=mask_sb[:], in_=mask_sb[:], compare_op=OP.is_gt, fill=1.0,
        base=0, pattern=[[-1, 128]], channel_multiplier=1,
    )
    nc.gpsimd.memset(ident48[:], 0.0)
    nc.gpsimd.affine_select(
        out=ident48[:], in_=ident48[:], compare_op=OP.not_equal, fill=1.0,
        base=0, pattern=[[-1, 48]], channel_multiplier=1,
    )
    nc.gpsimd.memset(ident1[:], 1.0)
    nc.vector.memset(zrow_sb[:], 0.0)
    nc.vector.memset(zcol_sb[:], 0.0)
    nc.vector.memset(epsn_sb[:], EPS_NORM)
    # absorb DMA/GPSIMD sem waits on DVE early (1-wait/instr HW limit)
    nc.vector.tensor_copy(out=touch_sb[:, 0:1], in_=mask_sb[:, 0:1])
    nc.vector.tensor_copy(out=touch_sb[:, 1:2], in_=vn_sb[:, 0, 0:1])
    # preload the exp/ln ACT table set early (hides the ~1.3us table load)
    nc.scalar.activation(touch_sb[:, 5:6], touch_sb[:, 0:1], AF.Exp)

    # ---------------- stage A: gates (one pass over xt) ----------------
    with tc.tile_pool(name="psA", bufs=1, space="PSUM") as psA:
        psg = psA.tile([2 * HPC, 2, 512], F32)
        for c in range(18):
            st, sp_ = (c == 0), (c == 17)
            for half in range(2):
                nc.tensor.matmul(
                    psg[:, half, :], lhsT=wt_sb[:, c, :],
                    rhs=xt_sb[:, c, half * 512:(half + 1) * 512],
                    start=st, stop=sp_)
        for half in range(2):
            hs = slice(half * 512, (half + 1) * 512)
            nc.scalar.activation(
                out=g12_sb[:, hs], in_=psg[:, half, :],
                func=AF.Identity, bias=bias_sb[:])

    # relayout [12, 1024] -> 2x [48, 128]  (partition c*6+h, c-major);
    # split across the two DMA queues (sync idle once inputs are done)
    for c in range(NCH):
        cs = slice(c * 128, (c + 1) * 128)
        nc.sync.dma_start(out=fg48[c * HPC:(c + 1) * HPC, :],
                          in_=g12_sb[0:HPC, cs])
        nc.gpsimd.dma_start(out=ig48[c * HPC:(c + 1) * HPC, :],
                            in_=g12_sb[HPC:2 * HPC, cs])

    # ---------------- stage A2: qk tiles staged during DMA window ----------
    with tc.tile_pool(name="psQK", bufs=4, space="PSUM") as psQK:
        for r in range(NCH):
            cs = slice(r * 128, (r + 1) * 128)
            for h in range(HPC):
                qc, kc, pb = h // 2, 6 + h // 2, (h % 2) * 64
                q_ap = xt_sb[pb:pb + 64, qc, cs]
                k_ap = xt_sb[pb:pb + 64, kc, cs]
                pqk = psQK.tile([128, 512], F32, name="pqk")
                nc.tensor.matmul(pqk[:, 0:128], lhsT=k_ap, rhs=q_ap,
                                 start=True, stop=True)
                nc.vector.tensor_tensor(
                    out=cp0_sb[:, r, h, :], in0=pqk[:, 0:128], in1=mask_sb[:],
                    op=OP.mult)

    # ---------------- stage B: gate scalar chain ----------------
    with tc.tile_pool(name="psT", bufs=1, space="PSUM") as psT:
        # log_sigmoid(fg) = -ln(1 + exp(-fg)); lp = cumsum per chunk + offset
        nc.scalar.activation(ef48[:], fg48[:], AF.Exp, scale=-1.0)
        nc.scalar.activation(sp48[:], ef48[:], AF.Ln, bias=1.0)
        nc.vector.tensor_tensor_scan(
            out=ic48[:], data0=sp48[:], data1=sp48[:], initial=0.0,
            op0=OP.add, op1=OP.bypass)
        # chunk offsets: off = lmask.T @ chunk_sums   (strict lower by chunk)
        ps_off = psT.tile([48, 512], F32, name="ps_off")
        nc.tensor.matmul(ps_off[:, 0:1], lhsT=lmask_sb[:], rhs=ic48[:, 127:128],
                         start=True, stop=True)
        off_sb = persist.tile([48, 1], F32)
        nc.vector.tensor_copy(out=off_sb[:], in_=ps_off[:, 0:1])
        nc.scalar.activation(lp48[:], ic48[:], AF.Identity, bias=off_sb[:])
        nc.vector.tensor_add(m48[:], ig48[:], lp48[:])
        # per-head global max cH: rowmax -> transpose -> segmented max ->
        # transpose -> broadcast matmul
        nc.vector.tensor_reduce(out=rmax[:], in_=m48[:],
                                axis=mybir.AxisListType.X, op=OP.max)
        ps_r = psT.tile([1, 512], F32, name="ps_r")
        nc.tensor.transpose(ps_r[:, 0:48], rmax[:], ident48[:])
        nc.vector.tensor_reduce(
            out=mx6[:], in_=ps_r[:, 0:48].rearrange("p (c h) -> p h c", h=HPC),
            axis=mybir.AxisListType.X, op=OP.max)
        ps_m6 = psT.tile([HPC, 512], F32, name="ps_m6")
        nc.tensor.transpose(ps_m6[:, 0:1], mx6[:], ident1[:])
        nc.scalar.copy(cH6[:], ps_m6[:, 0:1])
        ps_ch = psT.tile([48, 512], F32, name="ps_ch")
        nc.tensor.matmul(ps_ch[:, 0:1], lhsT=bc6_sb[:], rhs=cH6[:],
                         start=True, stop=True)
        nc.vector.tensor_scalar_mul(negb[:], ps_ch[:, 0:1], -1.0)
        nc.vector.tensor_scalar(out=negb8[:], in0=ps_ch[:, 0:1], scalar1=-1.0,
                                scalar2=LN8, op0=OP.mult, op1=OP.add)
        nc.scalar.activation(em48[:], m48[:], AF.Exp, bias=negb[:])
        nc.scalar.activation(g48[:], lp48[:], AF.Exp, bias=negb8[:])
        ps_e = psT.tile([128, 512], F32, name="ps_e")
        nc.tensor.transpose(ps_e[:, 0:48], em48[:], ident48[:])
        nc.scalar.copy(em_col[:], ps_e[:, 0:48])
        ps_g = psT.tile([128, 512], F32, name="ps_g")
        nc.tensor.transpose(ps_g[:, 0:48], g48[:], ident48[:])
        nc.scalar.copy(g_col[:], ps_g[:, 0:48])
        # preload the sqrt ACT table set before the first epilogue needs it
        nc.scalar.activation(touch_sb[:, 6:7], touch_sb[:, 0:1], AF.Sqrt)

    if os.environ.get("KBISECT") == "b":
        dump = persist.tile([128, GD], BF16)
        nc.vector.memset(dump[:], 0.0)
        nc.vector.tensor_copy(out=dump[:, 0:48], in_=em_col[:])
        nc.vector.tensor_copy(out=dump[:, 48:96], in_=g_col[:])
        for r in range(NCH):
            nc.sync.dma_start(out=out[r * 128:(r + 1) * 128, :], in_=dump[:])
        return

    emc = em_col.rearrange("p (c h) -> p c h", h=HPC)
    gc = g_col.rearrange("p (c h) -> p c h", h=HPC)

    # ---------------- stage C: chunked attention ----------------
    with (
        tc.tile_pool(name="psW", bufs=1, space="PSUM") as psW,
        tc.tile_pool(name="psH", bufs=2, space="PSUM") as psH,
        tc.tile_pool(name="work", bufs=3) as work,
        tc.tile_pool(name="wstate", bufs=2) as wstate,
        tc.tile_pool(name="hout", bufs=2) as hout,
    ):
        # head h's state at partitions (h%2)*64..+64, column h//2.
        # dummy zero matmul claims the shared bank once, then accumulate.
        psw_t = psW.tile([128, 512], F32)
        psum_W = psw_t[:, 0:3 * DA].rearrange("p (c d) -> p c d", d=DA)
        nc.tensor.matmul(psw_t[:, 0:3 * DA], lhsT=zcol_sb[:],
                         rhs=zrow_sb[:, 0:3 * DA], start=True, stop=True)

        for r in range(NCH):
            cs = slice(r * 128, (r + 1) * 128)
            if r > 0:
                w_sb = wstate.tile([128, HPC // 2, DA], BF16)
                nc.scalar.copy(w_sb[:], psum_W[:])

            # em folded into the qk tiles and into k (one broadcast TT each)
            em_b = em_col[:, r * HPC:(r + 1) * HPC]
            cpem = work.tile([128, HPC, 128], BF16, name="cpem")
            nc.vector.tensor_tensor(
                out=cpem[:], in0=cp0_sb[:, r, :, :],
                in1=em_b.broadcast_to((128, HPC, 128)), op=OP.mult)
            kpem = work.tile([128, HPC, DH], BF16, name="kpem")
            nc.gpsimd.tensor_tensor(
                out=kpem[:],
                in0=kn_sb[:, r, :].rearrange("p (h d) -> p h d", d=DH),
                in1=em_b.broadcast_to((128, HPC, DH)), op=OP.mult)

            h_sb = hout.tile([128, HPC, DH], BF16, name="h_sb")
            o_sb = hout.tile([128, GD], BF16, name="o_sb")

            ph_t = psH.tile([128, 512], F32, name="ph")
            ph = ph_t[:, 0:HPC * DA].rearrange("p (h d) -> p h d", d=DA)
            nc.tensor.matmul(ph_t[:, 0:HPC * DA], lhsT=zcol_sb[:],
                             rhs=zrow_sb[:], start=True, stop=True)

            for h in range(HPC):
                qc, pb = h // 2, (h % 2) * 64
                q_ap = xt_sb[pb:pb + 64, qc, cs]
                va_ap = vn_sb[:, r, h * DA:(h + 1) * DA]
                if r > 0:
                    nc.tensor.matmul(ph[:, h, :], lhsT=q_ap,
                                     rhs=w_sb[pb:pb + 64, h // 2, :],
                                     start=False, stop=False,
                                     skip_group_check=True)
                nc.tensor.matmul(ph[:, h, :], lhsT=cpem[:, h, :], rhs=va_ap,
                                 start=False, stop=True, skip_group_check=True)
                nc.tensor.matmul(
                    psum_W[pb:pb + 64, h // 2, :],
                    lhsT=kpem[:, h, :], rhs=va_ap,
                    start=False, stop=(r == NCH - 1),
                    tile_position=(0, pb), skip_group_check=True)

            # ---- epilogue: h = U / max(|R|, g); groupnorm over dh ----
            ab = work.tile([128, HPC], F32, name="ab")
            nc.scalar.activation(out=ab[:], in_=ph[:, :, DH], func=AF.Abs)
            den = work.tile([128, HPC], F32, name="den")
            nc.vector.tensor_tensor(out=den[:], in0=ab[:], in1=gc[:, r, :],
                                    op=OP.max)
            scl = work.tile([128, HPC], F32, name="scl")
            nc.vector.reciprocal(scl[:], den[:])
            # h = U * scl (all heads, one broadcast TT from PSUM)
            nc.vector.tensor_tensor(
                out=h_sb[:], in0=ph[:, :, 0:DH],
                in1=scl.broadcast_to((128, HPC, DH)), op=OP.mult)
            # mean = sumU_col * scl  (1/64 pre-folded into the vsum column)
            mean = work.tile([128, HPC], F32, name="mean")
            nc.vector.tensor_mul(mean[:], ph[:, :, DH + 1], scl[:])
            # var = sum(h^2)/64 - mean^2 ; rstd = 1/sqrt(var + eps)
            sq = work.tile([128, HPC, DH], F32, name="sq")
            nc.scalar.activation(
                out=sq.rearrange("p h d -> p (h d)"),
                in_=h_sb.rearrange("p h d -> p (h d)"), func=AF.Square)
            ssq = work.tile([128, HPC], F32, name="ssq")
            nc.vector.tensor_reduce(out=ssq[:], in_=sq[:],
                                    axis=mybir.AxisListType.X, op=OP.add)
            msq = work.tile([128, HPC], F32, name="msq")
            nc.vector.tensor_mul(msq[:], mean[:], mean[:])
            var = work.tile([128, HPC], F32, name="var")
            nc.vector.scalar_tensor_tensor(
                out=var[:], in0=ssq[:], scalar=1.0 / DH, in1=msq[:],
                op0=OP.mult, op1=OP.subtract)
            sd = work.tile([128, HPC], F32, name="sd")
            nc.scalar.activation(sd[:], var[:], AF.Sqrt, bias=epsn_sb[:])
            rstd = work.tile([128, HPC], F32, name="rstd")
            nc.vector.reciprocal(rstd[:], sd[:])
            mr = work.tile([128, HPC], F32, name="mr")
            nc.vector.tensor_mul(mr[:], mean[:], rstd[:])
            # out = h*rstd - mean*rstd  (two broadcast TTs on idle GPSIMD)
            t1 = hout.tile([128, HPC, DH], BF16, name="t1")
            nc.gpsimd.tensor_tensor(
                out=t1[:], in0=h_sb[:],
                in1=rstd.broadcast_to((128, HPC, DH)), op=OP.mult)
            nc.gpsimd.tensor_tensor(
                out=o_sb.rearrange("p (h d) -> p h d", d=DH), in0=t1[:],
                in1=mr.broadcast_to((128, HPC, DH)), op=OP.subtract)
            nc.sync.dma_start(out=out[cs, :], in_=o_sb[:])


_CACHED_NC = None


def _get_nc():
    global _CACHED_NC
    if _CACHED_NC is None:
        _CACHED_NC = build_nc()
    return _CACHED_NC


def _prep_core(q, k, v, igate_w, igate_b, fgate_w, fgate_b, b, g):
    """Build the per-core input dict.  Core = (batch b, head-group g).
    Features are permuted so this core's 6 heads come first in each of the
    q/k/v blocks (gate result is permutation invariant given matching wt)."""
    heads = list(range(6 * g, 6 * g + 6)) + list(range(6 * (1 - g), 6 * (1 - g) + 6))
    dperm = np.concatenate([np.arange(h * DH, (h + 1) * DH) for h in heads])
    qp = q[b][:, dperm]
    kp = k[b][:, dperm]
    vp = v[b][:, dperm]
    xt = np.ascontiguousarray(
        np.concatenate([qp, kp, vp], axis=1).T).astype(ml_dtypes.bfloat16)
    kn = np.ascontiguousarray(kp[:, :GD]).astype(ml_dtypes.bfloat16)
    va = np.ones((S, HPC, DA), np.float32)
    vs = vp[:, :GD].reshape(S, HPC, DH)
    va[:, :, :DH] = vs
    va[:, :, DH + 1] = vs.sum(-1) / DH   # mean column (1/64 pre-folded)
    vn = np.ascontiguousarray(va.reshape(S, HPC * DA)).astype(ml_dtypes.bfloat16)

    fperm = np.concatenate([dperm, dperm + DIM, dperm + 2 * DIM])
    hsel = heads[:HPC]
    wfg = fgate_w[hsel][:, fperm]          # (6, 2304) - fg first (rows 0-5)
    wig = igate_w[hsel][:, fperm]
    wT = np.concatenate([wfg.T, wig.T], axis=1)      # (2304, 12)
    wt_host = np.ascontiguousarray(
        wT.reshape(18, 128, 2 * HPC).transpose(1, 0, 2).reshape(128, -1)
    ).astype(ml_dtypes.bfloat16)
    bias_host = np.concatenate([fgate_b[hsel], igate_b[hsel]]).reshape(-1, 1)

    # lmask[j=(c',h), i=(c,h)] = 1 if same head and c' < c (chunk offsets);
    # partition index is c-major: p = c*6 + h
    hh = np.arange(48) % HPC
    cc = np.arange(48) // HPC
    lmask = ((hh[:, None] == hh[None, :]) &
             (cc[:, None] < cc[None, :])).astype(np.float32)
    bc6 = (np.arange(HPC)[:, None] == hh[None, :]).astype(np.float32)
    return {"xt": xt, "kn": kn, "vn": vn, "wt": wt_host,
            "bias": np.ascontiguousarray(bias_host.astype(np.float32)),
            "lmask": lmask, "bc6": bc6}, dperm[:GD]


_LAST_RESULT = {}


def kernel(q, k, v, igate_w, igate_b, fgate_w, fgate_b, norm_w, norm_b,
           **run_kwargs):
    nc = _get_nc()
    in_maps, slots = [], []
    for core in range(8):
        b, g = core // 2, core % 2
        im, dsel = _prep_core(q, k, v, igate_w, igate_b, fgate_w, fgate_b, b, g)
        in_maps.append(im)
        slots.append((b, dsel))

    res = run_bass_kernel_spmd(nc, in_maps, core_ids=list(range(8)),
                               **run_kwargs)
    _LAST_RESULT["res"] = res

    out = np.zeros((B, S, DIM), np.float32)
    for core in range(8):
        b, dsel = slots[core]
        out[b][:, dsel] = res.results[core]["out"].astype(np.float32)

    # the reference's affine (residual weight / bias) on the normed output;
    # identity when norm_w/norm_b are zero (cheap host epilogue otherwise)
    if np.any(norm_w) or np.any(norm_b):
        out = out * (1.0 + norm_w)[None, None, :] + norm_b[None, None, :]
    return out


# revision 47
# speedup vs baseline: 1.0917x; 1.0917x over previous
"""Trainium2 Bass kernel for nn_MatrixLSTMCell (mLSTM, parallel stabilized form).

Sharding: 48 (b, head) pairs across 8 cores -> each core handles one batch b
and a group of 6 heads (2 cores per batch).

Algorithm (equivalent to the reference up to fp tolerance):
  lp[t]  = cumsum(softplus(-fg))[t]      (= -L[t])
  m[t]   = ig[t] + lp[t];  cH = max_t m  (per head)
  em[j]  = exp(m[j] - cH)
  g[i]   = exp(lp[i] - cH + ln 8)        (ln 8 = ln sqrt(dh))
  U[i]   = sum_{j<=i} (q_i.k_j) em[j] [v|1|vsum][j]   (chunked linear attn)
  R[i]   = U's ones-column (rowsum of weights), sumU = U's vsum-column
  h[i]   = U / max(|R|, g)               (exactly the reference normalizer:
           the global 1/sqrt(dh) and the per-row exp(cH - M[i]) factors
           cancel inside max(), and the +eps term is provably negligible)
  out    = (h - mean) / sqrt(var + 1e-5) per (i, head) over dh=64
The ones/vsum columns make rowsum and mean free (matmul byproducts).
The causal sum splits into 128-row chunks: tril-masked qk tiles (staged in
SBUF during the input-DMA window) times (va*em), plus a running state
W = sum_j k[j] em[j] va[j]^T applied as q @ W.
"""

import math
import os

import numpy as np
import ml_dtypes

import concourse.bass as bass
import concourse.bacc as bacc
import concourse.mybir as mybir
import concourse.tile as tile
from concourse.bass_utils import run_bass_kernel_spmd

F32 = mybir.dt.float32
BF16 = mybir.dt.bfloat16

B, S, DIM = 4, 1024, 768
NH, DH = 12, 64
HPC = 6            # heads per core
GD = HPC * DH      # 384 output dims per core
DA = DH + 2        # v augmented with ones and vsum columns
NCH = S // 128     # 8 chunks
EPS_NORM = 1e-5
LN8 = math.log(8.0)
AF = mybir.ActivationFunctionType
OP = mybir.AluOpType


def build_nc():
    # Bacc (not raw Bass): its compile() pass splits multi-sem waits into
    # standalone event-semaphore instructions (TRN2 allows 1 wait/instr).
    nc = bacc.Bacc(None, target_bir_lowering=False)

    xt = nc.dram_tensor("xt", [3 * DIM, S], BF16, kind="ExternalInput")[:]
    kn = nc.dram_tensor("kn", [S, GD], BF16, kind="ExternalInput")[:]
    vn = nc.dram_tensor("vn", [S, HPC * DA], BF16, kind="ExternalInput")[:]
    wt = nc.dram_tensor("wt", [128, 18 * 2 * HPC], BF16, kind="ExternalInput")[:]
    bias = nc.dram_tensor("bias", [2 * HPC, 1], F32, kind="ExternalInput")[:]
    lmask = nc.dram_tensor("lmask", [48, 48], F32, kind="ExternalInput")[:]
    bc6 = nc.dram_tensor("bc6", [HPC, 48], F32, kind="ExternalInput")[:]
    out = nc.dram_tensor("out", [S, GD], BF16, kind="ExternalOutput")[:]

    with tile.TileContext(nc) as tc:
        with tc.tile_pool(name="persist", bufs=1) as persist:
            _body(nc, tc, persist, xt, kn, vn, wt, bias, lmask, bc6, out)
    nc.finalize()
    return nc


def _body(nc, tc, persist, xt, kn, vn, wt, bias, lmask, bc6, out):
    # ---------------- persistent SBUF ----------------
    xt_sb = persist.tile([128, 18, S], BF16)
    kn_sb = persist.tile([128, NCH, GD], BF16)
    vn_sb = persist.tile([128, NCH, HPC * DA], BF16)
    wt_sb = persist.tile([128, 18, 2 * HPC], BF16)
    bias_sb = persist.tile([2 * HPC, 1], F32)
    lmask_sb = persist.tile([48, 48], F32)
    bc6_sb = persist.tile([HPC, 48], F32)
    mask_sb = persist.tile([128, 128], BF16)     # 1.0 where j<=i else 0
    ident48 = persist.tile([48, 48], F32)
    ident1 = persist.tile([1, 1], F32)
    touch_sb = persist.tile([128, 8], F32)
    epsn_sb = persist.tile([128, 1], F32)
    zcol_sb = persist.tile([1, 128], BF16)
    zwide_sb = persist.tile([1, 512], BF16)

    g12_sb = persist.tile([2 * HPC, S], F32)     # rows 0-5 fg, 6-11 ig
    fg48 = persist.tile([48, 128], F32)          # partition h*8+c
    ig48 = persist.tile([48, 128], F32)
    ef48 = persist.tile([48, 128], F32)
    sp48 = persist.tile([48, 128], F32)
    ic48 = persist.tile([48, 128], F32)          # intra-chunk cumsum
    lp48 = persist.tile([48, 128], F32)
    m48 = persist.tile([48, 128], F32)
    rmax = persist.tile([48, 1], F32)
    mx6 = persist.tile([1, HPC], F32)
    cH6 = persist.tile([HPC, 1], F32)
    negb = persist.tile([48, 1], F32)            # -cH per (h,c) partition
    negb8 = persist.tile([48, 1], F32)           # -cH + ln 8
    em48 = persist.tile([48, 128], F32)
    g48 = persist.tile([48, 128], F32)
    em_col = persist.tile([128, 48], BF16)       # [i, c*6+h]
    g_col = persist.tile([128, 48], F32)

    cp0_sb = persist.tile([128, NCH, HPC, 128], BF16)   # tril(qk) staged

    xt_c = xt.rearrange("(c p) s -> c p s", p=128)
    kn_c = kn.rearrange("(r p) d -> p r d", p=128)
    vn_c = vn.rearrange("(r p) d -> p r d", p=128)

    # ---------------- loads (wt/bias first: gates need them) ----------------
    nc.sync.dma_start(out=wt_sb[:], in_=wt.rearrange("p (c j) -> p c j", c=18))
    nc.sync.dma_start(out=bias_sb[:], in_=bias)
    nc.sync.dma_start(out=lmask_sb[:], in_=lmask)
    nc.sync.dma_start(out=bc6_sb[:], in_=bc6)
    for c in range(18):
        nc.sync.dma_start(out=xt_sb[:, c, :], in_=xt_c[c])
    nc.sync.dma_start(out=kn_sb[:], in_=kn_c)
    nc.sync.dma_start(out=vn_sb[:], in_=vn_c)

    # ---------------- constants ----------------
    nc.gpsimd.memset(mask_sb[:], 0.0)
    nc.gpsimd.affine_select(
        out=mask_sb[:], in_=mask_sb[:], compare_op=OP.is_gt, fill=1.0,
        base=0, pattern=[[-1, 128]], channel_multiplier=1,
    )
    nc.gpsimd.memset(ident48[:], 0.0)
    nc.gpsimd.affine_select(
        out=ident48[:], in_=ident48[:], compare_op=OP.not_equal, fill=1.0,
        base=0, pattern=[[-1, 48]], channel_multiplier=1,
    )
    nc.gpsimd.memset(ident1[:], 1.0)
    nc.vector.memset(zcol_sb[:], 0.0)
    nc.vector.memset(zwide_sb[:], 0.0)
    nc.vector.memset(epsn_sb[:], EPS_NORM)
    # absorb DMA/GPSIMD sem waits on DVE early (1-wait/instr HW limit)
    nc.vector.tensor_copy(out=touch_sb[:, 0:1], in_=mask_sb[:, 0:1])
    nc.vector.tensor_copy(out=touch_sb[:, 1:2], in_=vn_sb[:, 0, 0:1])
    # preload the exp/ln ACT table set early (hides the ~1.3us table load)
    nc.scalar.activation(touch_sb[:, 5:6], touch_sb[:, 0:1], AF.Exp)

    # ---------------- PE warm-up: ~4us of dummy matmuls flips HAM to 8/8
    # before the gate matmuls start (cold PE runs at 1.2 instead of 2.4 GHz)
    with tc.tile_pool(name="psWm", bufs=1, space="PSUM") as psWm:
        wmps = psWm.tile([128, 512], F32)
        for _ in range(10):
            nc.tensor.matmul(wmps[:], lhsT=zcol_sb[:], rhs=zwide_sb[:],
                             start=True, stop=True)

    # ---------------- stage A: gates (one pass over xt) ----------------
    with tc.tile_pool(name="psA", bufs=1, space="PSUM") as psA:
        psg = psA.tile([2 * HPC, 2, 512], F32)
        for c in range(18):
            st, sp_ = (c == 0), (c == 17)
            for half in range(2):
                nc.tensor.matmul(
                    psg[:, half, :], lhsT=wt_sb[:, c, :],
                    rhs=xt_sb[:, c, half * 512:(half + 1) * 512],
                    start=st, stop=sp_)
        for half in range(2):
            hs = slice(half * 512, (half + 1) * 512)
            nc.scalar.activation(
                out=g12_sb[:, hs], in_=psg[:, half, :],
                func=AF.Identity, bias=bias_sb[:])

    # relayout [12, 1024] -> 2x [48, 128]  (partition c*6+h, c-major);
    # split across the two DMA queues (sync idle once inputs are done)
    for c in range(NCH):
        cs = slice(c * 128, (c + 1) * 128)
        nc.sync.dma_start(out=fg48[c * HPC:(c + 1) * HPC, :],
                          in_=g12_sb[0:HPC, cs])
        nc.gpsimd.dma_start(out=ig48[c * HPC:(c + 1) * HPC, :],
                            in_=g12_sb[HPC:2 * HPC, cs])

    # ---------------- stage A2: qk tiles staged during DMA window ----------
    with tc.tile_pool(name="psQK", bufs=4, space="PSUM") as psQK:
        for r in range(NCH):
            cs = slice(r * 128, (r + 1) * 128)
            for h in range(HPC):
                qc, kc, pb = h // 2, 6 + h // 2, (h % 2) * 64
                q_ap = xt_sb[pb:pb + 64, qc, cs]
                k_ap = xt_sb[pb:pb + 64, kc, cs]
                pqk = psQK.tile([128, 512], F32, name="pqk")
                nc.tensor.matmul(pqk[:, 0:128], lhsT=k_ap, rhs=q_ap,
                                 start=True, stop=True)
                nc.vector.tensor_tensor(
                    out=cp0_sb[:, r, h, :], in0=pqk[:, 0:128], in1=mask_sb[:],
                    op=OP.mult)

    # ---------------- stage B: gate scalar chain ----------------
    with tc.tile_pool(name="psT", bufs=1, space="PSUM") as psT:
        # log_sigmoid(fg) = -ln(1 + exp(-fg)); lp = cumsum per chunk + offset
        nc.scalar.activation(ef48[:], fg48[:], AF.Exp, scale=-1.0)
        nc.scalar.activation(sp48[:], ef48[:], AF.Ln, bias=1.0)
        nc.vector.tensor_tensor_scan(
            out=ic48[:], data0=sp48[:], data1=sp48[:], initial=0.0,
            op0=OP.add, op1=OP.bypass)
        # chunk offsets: off = lmask.T @ chunk_sums   (strict lower by chunk)
        ps_off = psT.tile([48, 512], F32, name="ps_off")
        nc.tensor.matmul(ps_off[:, 0:1], lhsT=lmask_sb[:], rhs=ic48[:, 127:128],
                         start=True, stop=True)
        off_sb = persist.tile([48, 1], F32)
        nc.vector.tensor_copy(out=off_sb[:], in_=ps_off[:, 0:1])
        nc.scalar.activation(lp48[:], ic48[:], AF.Identity, bias=off_sb[:])
        nc.vector.tensor_add(m48[:], ig48[:], lp48[:])
        # per-head global max cH: rowmax -> transpose -> segmented max ->
        # transpose -> broadcast matmul
        nc.vector.tensor_reduce(out=rmax[:], in_=m48[:],
                                axis=mybir.AxisListType.X, op=OP.max)
        ps_r = psT.tile([1, 512], F32, name="ps_r")
        nc.tensor.transpose(ps_r[:, 0:48], rmax[:], ident48[:])
        nc.vector.tensor_reduce(
            out=mx6[:], in_=ps_r[:, 0:48].rearrange("p (c h) -> p h c", h=HPC),
            axis=mybir.AxisListType.X, op=OP.max)
        ps_m6 = psT.tile([HPC, 512], F32, name="ps_m6")
        nc.tensor.transpose(ps_m6[:, 0:1], mx6[:], ident1[:])
        nc.scalar.copy(cH6[:], ps_m6[:, 0:1])
        ps_ch = psT.tile([48, 512], F32, name="ps_ch")
        nc.tensor.matmul(ps_ch[:, 0:1], lhsT=bc6_sb[:], rhs=cH6[:],
                         start=True, stop=True)
        nc.vector.tensor_scalar_mul(negb[:], ps_ch[:, 0:1], -1.0)
        nc.vector.tensor_scalar(out=negb8[:], in0=ps_ch[:, 0:1], scalar1=-1.0,
                                scalar2=LN8, op0=OP.mult, op1=OP.add)
        nc.scalar.activation(em48[:], m48[:], AF.Exp, bias=negb[:])
        nc.scalar.activation(g48[:], lp48[:], AF.Exp, bias=negb8[:])
        ps_e = psT.tile([128, 512], F32, name="ps_e")
        nc.tensor.transpose(ps_e[:, 0:48], em48[:], ident48[:])
        nc.scalar.copy(em_col[:], ps_e[:, 0:48])
        ps_g = psT.tile([128, 512], F32, name="ps_g")
        nc.tensor.transpose(ps_g[:, 0:48], g48[:], ident48[:])
        nc.scalar.copy(g_col[:], ps_g[:, 0:48])
        # preload the sqrt ACT table set before the first epilogue needs it
        nc.scalar.activation(touch_sb[:, 6:7], touch_sb[:, 0:1], AF.Sqrt)
        # keep-alive matmuls: bridge the PE-idle gap at the end of the gate
        # chain so HAM does not re-throttle before stage C
        for _ in range(6):
            nc.tensor.matmul(ps_g[:, 64:512], lhsT=zcol_sb[:],
                             rhs=zwide_sb[:, 0:448], start=True, stop=True)

    if os.environ.get("KBISECT") == "b":
        dump = persist.tile([128, GD], BF16)
        nc.vector.memset(dump[:], 0.0)
        nc.vector.tensor_copy(out=dump[:, 0:48], in_=em_col[:])
        nc.vector.tensor_copy(out=dump[:, 48:96], in_=g_col[:])
        for r in range(NCH):
            nc.sync.dma_start(out=out[r * 128:(r + 1) * 128, :], in_=dump[:])
        return

    emc = em_col.rearrange("p (c h) -> p c h", h=HPC)
    gc = g_col.rearrange("p (c h) -> p c h", h=HPC)

    # ---------------- stage C: chunked attention ----------------
    with (
        tc.tile_pool(name="psW", bufs=1, space="PSUM") as psW,
        tc.tile_pool(name="psH", bufs=2, space="PSUM") as psH,
        tc.tile_pool(name="work", bufs=3) as work,
        tc.tile_pool(name="wstate", bufs=2) as wstate,
        tc.tile_pool(name="hout", bufs=2) as hout,
    ):
        # head h's state at partitions (h%2)*64..+64, column h//2.
        # dummy zero matmul claims the shared bank once, then accumulate.
        psw_t = psW.tile([128, 512], F32)
        psum_W = psw_t[:, 0:3 * DA].rearrange("p (c d) -> p c d", d=DA)
        nc.tensor.matmul(psw_t[:, 0:3 * DA], lhsT=zcol_sb[:],
                         rhs=zwide_sb[:, 0:3 * DA], start=True, stop=True)

        for r in range(NCH):
            cs = slice(r * 128, (r + 1) * 128)
            if r > 0:
                w_sb = wstate.tile([128, HPC // 2, DA], BF16)
                nc.scalar.copy(w_sb[:], psum_W[:])

            # em folded into the qk tiles and into k (one broadcast TT each)
            em_b = em_col[:, r * HPC:(r + 1) * HPC]
            cpem = work.tile([128, HPC, 128], BF16, name="cpem")
            nc.vector.tensor_tensor(
                out=cpem[:], in0=cp0_sb[:, r, :, :],
                in1=em_b.broadcast_to((128, HPC, 128)), op=OP.mult)
            kpem = work.tile([128, HPC, DH], BF16, name="kpem")
            nc.gpsimd.tensor_tensor(
                out=kpem[:],
                in0=kn_sb[:, r, :].rearrange("p (h d) -> p h d", d=DH),
                in1=em_b.broadcast_to((128, HPC, DH)), op=OP.mult)

            h_sb = hout.tile([128, HPC, DH], BF16, name="h_sb")
            o_sb = hout.tile([128, GD], BF16, name="o_sb")

            ph_t = psH.tile([128, 512], F32, name="ph")
            ph = ph_t[:, 0:HPC * DA].rearrange("p (h d) -> p h d", d=DA)
            nc.tensor.matmul(ph_t[:, 0:HPC * DA], lhsT=zcol_sb[:],
                             rhs=zwide_sb[:, 0:HPC * DA], start=True,
                             stop=True)

            for h in range(HPC):
                qc, pb = h // 2, (h % 2) * 64
                va_ap = vn_sb[:, r, h * DA:(h + 1) * DA]
                if r > 0:
                    nc.tensor.matmul(ph[:, h, :],
                                     lhsT=xt_sb[pb:pb + 64, qc, cs],
                                     rhs=w_sb[pb:pb + 64, h // 2, :],
                                     start=False, stop=False,
                                     skip_group_check=True)
                nc.tensor.matmul(ph[:, h, :], lhsT=cpem[:, h, :], rhs=va_ap,
                                 start=False, stop=True, skip_group_check=True)
                nc.tensor.matmul(
                    psum_W[pb:pb + 64, h // 2, :],
                    lhsT=kpem[:, h, :], rhs=va_ap,
                    start=False, stop=(r == NCH - 1),
                    tile_position=(0, pb), skip_group_check=True)

            # ---- epilogue: h = U / max(|R|, g); groupnorm over dh ----
            ab = work.tile([128, HPC], F32, name="ab")
            nc.scalar.activation(out=ab[:], in_=ph[:, :, DH], func=AF.Abs)
            den = work.tile([128, HPC], F32, name="den")
            nc.vector.tensor_tensor(out=den[:], in0=ab[:], in1=gc[:, r, :],
                                    op=OP.max)
            scl = work.tile([128, HPC], F32, name="scl")
            nc.vector.reciprocal(scl[:], den[:])
            # h = U * scl (all heads, one broadcast TT from PSUM)
            nc.vector.tensor_tensor(
                out=h_sb[:], in0=ph[:, :, 0:DH],
                in1=scl.broadcast_to((128, HPC, DH)), op=OP.mult)
            # mean = sumU_col * scl  (1/64 pre-folded into the vsum column)
            mean = work.tile([128, HPC], F32, name="mean")
            nc.vector.tensor_mul(mean[:], ph[:, :, DH + 1], scl[:])
            # var = sum(h^2)/64 - mean^2 ; rstd = 1/sqrt(var + eps)
            sq = work.tile([128, HPC, DH], F32, name="sq")
            nc.scalar.activation(
                out=sq.rearrange("p h d -> p (h d)"),
                in_=h_sb.rearrange("p h d -> p (h d)"), func=AF.Square)
            ssq = work.tile([128, HPC], F32, name="ssq")
            nc.vector.tensor_reduce(out=ssq[:], in_=sq[:],
                                    axis=mybir.AxisListType.X, op=OP.add)
            msq = work.tile([128, HPC], F32, name="msq")
            nc.vector.tensor_mul(msq[:], mean[:], mean[:])
            var = work.tile([128, HPC], F32, name="var")
            nc.vector.scalar_tensor_tensor(
                out=var[:], in0=ssq[:], scalar=1.0 / DH, in1=msq[:],
                op0=OP.mult, op1=OP.subtract)
            sd = work.tile([128, HPC], F32, name="sd")
            nc.scalar.activation(sd[:], var[:], AF.Sqrt, bias=epsn_sb[:])
            rstd = work.tile([128, HPC], F32, name="rstd")
            nc.vector.reciprocal(rstd[:], sd[:])
            mr = work.tile([128, HPC], F32, name="mr")
            nc.vector.tensor_mul(mr[:], mean[:], rstd[:])
            # out = h*rstd - mean*rstd  (two broadcast TTs on idle GPSIMD)
            t1 = hout.tile([128, HPC, DH], BF16, name="t1")
            nc.gpsimd.tensor_tensor(
                out=t1[:], in0=h_sb[:],
                in1=rstd.broadcast_to((128, HPC, DH)), op=OP.mult)
            nc.gpsimd.tensor_tensor(
                out=o_sb.rearrange("p (h d) -> p h d", d=DH), in0=t1[:],
                in1=mr.broadcast_to((128, HPC, DH)), op=OP.subtract)
            nc.sync.dma_start(out=out[cs, :], in_=o_sb[:])


_CACHED_NC = None


def _get_nc():
    global _CACHED_NC
    if _CACHED_NC is None:
        _CACHED_NC = build_nc()
    return _CACHED_NC


def _prep_core(q, k, v, igate_w, igate_b, fgate_w, fgate_b, b, g):
    """Build the per-core input dict.  Core = (batch b, head-group g).
    Features are permuted so this core's 6 heads come first in each of the
    q/k/v blocks (gate result is permutation invariant given matching wt)."""
    heads = list(range(6 * g, 6 * g + 6)) + list(range(6 * (1 - g), 6 * (1 - g) + 6))
    dperm = np.concatenate([np.arange(h * DH, (h + 1) * DH) for h in heads])
    qp = q[b][:, dperm]
    kp = k[b][:, dperm]
    vp = v[b][:, dperm]
    xt = np.ascontiguousarray(
        np.concatenate([qp, kp, vp], axis=1).T).astype(ml_dtypes.bfloat16)
    kn = np.ascontiguousarray(kp[:, :GD]).astype(ml_dtypes.bfloat16)
    va = np.ones((S, HPC, DA), np.float32)
    vs = vp[:, :GD].reshape(S, HPC, DH)
    va[:, :, :DH] = vs
    va[:, :, DH + 1] = vs.sum(-1) / DH   # mean column (1/64 pre-folded)
    vn = np.ascontiguousarray(va.reshape(S, HPC * DA)).astype(ml_dtypes.bfloat16)

    fperm = np.concatenate([dperm, dperm + DIM, dperm + 2 * DIM])
    hsel = heads[:HPC]
    wfg = fgate_w[hsel][:, fperm]          # (6, 2304) - fg first (rows 0-5)
    wig = igate_w[hsel][:, fperm]
    wT = np.concatenate([wfg.T, wig.T], axis=1)      # (2304, 12)
    wt_host = np.ascontiguousarray(
        wT.reshape(18, 128, 2 * HPC).transpose(1, 0, 2).reshape(128, -1)
    ).astype(ml_dtypes.bfloat16)
    bias_host = np.concatenate([fgate_b[hsel], igate_b[hsel]]).reshape(-1, 1)

    # lmask[j=(c',h), i=(c,h)] = 1 if same head and c' < c (chunk offsets);
    # partition index is c-major: p = c*6 + h
    hh = np.arange(48) % HPC
    cc = np.arange(48) // HPC
    lmask = ((hh[:, None] == hh[None, :]) &
             (cc[:, None] < cc[None, :])).astype(np.float32)
    bc6 = (np.arange(HPC)[:, None] == hh[None, :]).astype(np.float32)
    return {"xt": xt, "kn": kn, "vn": vn, "wt": wt_host,
            "bias": np.ascontiguousarray(bias_host.astype(np.float32)),
            "lmask": lmask, "bc6": bc6}, dperm[:GD]


_LAST_RESULT = {}


def kernel(q, k, v, igate_w, igate_b, fgate_w, fgate_b, norm_w, norm_b,
           **run_kwargs):
    nc = _get_nc()
    in_maps, slots = [], []
    for core in range(8):
        b, g = core // 2, core % 2
        im, dsel = _prep_core(q, k, v, igate_w, igate_b, fgate_w, fgate_b, b, g)
        in_maps.append(im)
        slots.append((b, dsel))

    res = run_bass_kernel_spmd(nc, in_maps, core_ids=list(range(8)),
                               **run_kwargs)
    _LAST_RESULT["res"] = res

    out = np.zeros((B, S, DIM), np.float32)
    for core in range(8):
        b, dsel = slots[core]
        out[b][:, dsel] = res.results[core]["out"].astype(np.float32)

    # the reference's affine (residual weight / bias) on the normed output;
    # identity when norm_w/norm_b are zero (cheap host epilogue otherwise)
    if np.any(norm_w) or np.any(norm_b):
        out = out * (1.0 + norm_w)[None, None, :] + norm_b[None, None, :]
    return out
